# revision 26
# baseline (speedup 1.0000x reference)
"""Differential attention Trainium2 kernel (Bass/Tile), 8-core SPMD.

reference:
  attn1 = softmax(causal(Q1 K1^T / sqrt(D))) V
  attn2 = softmax(causal(Q2 K2^T / sqrt(D))) V
  out   = attn1 - exp(lambda_log) * attn2
shapes: [B=2, H=12, S=2048, D=128] fp32.

Sharding: B*H = 24 head-batches, 3 per NeuronCore (data/head parallel, no
cross-core comms). Host pre-transposes Q/K to [D, S] layout so the device
needs no on-chip transposes; device returns output d-major ([D, S] per
head) and the host transposes back.

Matmul dtype strategy: all matmul operands (Q^T, K^T, V, exp-scores E,
ones) are fp16 (PE streams 1 col/cycle for 2-byte dtypes; fp32 is 4x
slower). PSUM accumulation stays fp32, lambda applied exactly in fp32.
Error ~4e-4 of output absmax.

Device algorithm per (head, pass), in score-transposed layout:
  S_T[k, q] = matmul(lhsT=K^T_j, rhs=Q^T[q-group])      (contract D)
  E_T = exp(SCALE * S_T)  fp16   (ScalarE, PSUM->SBUF)
  out_T[d, q] += matmul(lhsT=V_j, rhs=E_T)              (contract k, PSUM acc)
  sums[128, q] += matmul(lhsT=ones128, rhs=E_T)         (denominator,
                                  pre-broadcast across all partitions)
then fin = out1_T*recip(sums1) - lam*(out2_T*recip(sums2)) on DVE.

Perf structure (v3):
  - score tiles are [128,1024] fp32 = 2 PSUM banks; full key-tiles are
    processed in j-PAIRS with a single exp per pair (ScalarE has ~260ns
    fixed cost per instruction). The 4 diagonal tiles are packed into two
    tiles, (dr0|dr1) and (dr2|dr3), each with ONE exp and ONE two-band
    copy_predicated (uniform band stride inside each packed tile).
  - ONE flat software pipeline across all (head, group) work: each unit
    (score-tile, pass) is emitted in two stages -- stage1 = QK matmuls
    [+ band mask] + exp, stage2 = sums/PV matmuls -- with a global lag of
    4 units between them.  This keeps ready matmuls in the PE's strict
    FIFO while exps are in flight on ScalarE, and (crucially) emits the
    next group's copy_predicated ops on the strict-FIFO Vector engine
    BEFORE the previous group's ~5us reciprocal/normalize epilogue, so
    the exp chain is never blocked behind the epilogue at group/head
    boundaries (this was a measured 4us PE stall per head + PE clock
    re-throttle).
  - per-pass epilogue halves (reciprocal + multiply) are emitted as soon
    as that pass's accumulation stops, shortening the serial tail.
  - per-head DMA loads are critical-first (k[0:128]+q[0:512] land first,
    so the first QK can start ~3us into the kernel) and the next head's
    loads are hoisted to the start of the previous head's last group.
"""

import sys

sys.path.insert(0, "/opt/trn_rl_repo")

import numpy as np

B, H, S, D = 2, 12, 2048, 128
NCORES = 8
BH = B * H
HEADS = BH // NCORES  # 3 heads per core
P = 128
NT = S // P           # 16 key tiles
GW = 512              # query-group width (matmul free dim)
G = S // GW           # 4 query groups
TPG = GW // P         # 4 tiles per group
SCALE = float(D) ** -0.5
LAG = 4               # software-pipeline depth, in (tile, pass) units
# fp8e4m3 E/V + DoubleRow PE packing for full key-tiles: measured NET LOSS
# (DoubleRow matmuls stream no faster than the two fp16 matmuls they
# replace, and fp8 activation output slows ScalarE by ~20%); keep off.
USE_FP8 = False
V8_PAIRS = TPG * (G - 1) // 2  # key-tile pairs ever used by full tiles
# constant shift inside every exp: E = exp(scale*s - EXPB).  Softmax is
# shift-invariant (numerator and denominator scale together); keeps the
# max E (~exp(6)) inside fp8e4m3 range (max 448) with wide margin.
EXPB = 2.0794415416798357  # ln(8)

_PROGRAM = None


def _build_program():
    import concourse.mybir as mybir
    import concourse.tile as tile
    from concourse import bacc

    fp32 = mybir.dt.float32
    fp16 = mybir.dt.float16
    fp8 = mybir.dt.float8e4
    u8 = mybir.dt.uint8
    Exp = mybir.ActivationFunctionType.Exp
    Mult = mybir.AluOpType.mult
    Add = mybir.AluOpType.add
    DR = mybir.MatmulPerfMode.DoubleRow

    nc = bacc.Bacc(None)
    qt1 = nc.dram_tensor("qt1", [HEADS, P, S], fp16, kind="ExternalInput")
    kt1 = nc.dram_tensor("kt1", [HEADS, P, S], fp16, kind="ExternalInput")
    qt2 = nc.dram_tensor("qt2", [HEADS, P, S], fp16, kind="ExternalInput")
    kt2 = nc.dram_tensor("kt2", [HEADS, P, S], fp16, kind="ExternalInput")
    vd = nc.dram_tensor("v", [HEADS, P, NT, D], fp16, kind="ExternalInput")
    neglam = nc.dram_tensor("neglam", [P, 1], fp32, kind="ExternalInput")
    onesd = nc.dram_tensor("ones", [P, P], fp16, kind="ExternalInput")
    tri = nc.dram_tensor("tri", [P, P], u8, kind="ExternalInput")
    if USE_FP8:
        v8d = nc.dram_tensor(
            "v8", [HEADS, P, V8_PAIRS, 2, D], fp8, kind="ExternalInput"
        )
        ones8d = nc.dram_tensor("ones8", [P, 2, P], fp8, kind="ExternalInput")
    out = nc.dram_tensor("out", [HEADS, P, S], fp32, kind="ExternalOutput")

    with tile.TileContext(nc) as tc:
        with (
            tc.tile_pool(name="const", bufs=1) as cpool,
            tc.tile_pool(name="load", bufs=3) as lpool,
            tc.tile_pool(name="et", bufs=4) as epool,
            tc.tile_pool(name="fin", bufs=4) as fpool,
            tc.tile_pool(name="spsum", bufs=1, space="PSUM") as spool,
            tc.tile_pool(name="opsum", bufs=1, space="PSUM") as opool,
            tc.tile_pool(name="supsum", bufs=1, space="PSUM") as upool,
        ):
            tri_s = cpool.tile([P, P], u8)
            negbig = cpool.tile([P, P], fp32)
            nc.vector.memset(negbig[:], -1.0e30)
            neglam_s = cpool.tile([P, 1], fp32)
            ones_mat = cpool.tile([P, P], fp16)
            ones8_s = (
                cpool.tile([P, 2, P], fp8, name="ones8_s") if USE_FP8 else None
            )
            expb_s = cpool.tile([P, 1], fp32)
            nc.vector.memset(expb_s[:], -EXPB)
            # prewarm the ScalarE exp table: the lazy ACT_TABLE_LOAD costs
            # ~1.3us and would otherwise delay the first real exp
            warm = cpool.tile([P, 1], fp32)
            nc.scalar.activation(warm[:], expb_s[:], Exp)

            head_tiles = {}

            def emit_loads(h):
                # Sync (HWDGE) queue issue costs ~600ns per dma_start, so
                # only the startup-critical loads go there; everything
                # prefetchable is issued via the idle GpSimd SWDGE queue.
                qk = [
                    lpool.tile([P, S], fp16, tag=n, name=f"{n}_{h}")
                    for n in ("q1", "k1", "q2", "k2")
                ]
                v_s = lpool.tile([P, NT, D], fp16, tag="v", name=f"v_{h}")
                v8_s = None
                if USE_FP8:
                    v8_s = lpool.tile(
                        [P, V8_PAIRS, 2, D], fp8, tag="v8", name=f"v8_{h}"
                    )
                if h == 0:
                    nc.sync.dma_start(qk[1][:, 0:GW], kt1[h][:, 0:GW])
                    nc.sync.dma_start(qk[0][:, 0:GW], qt1[h][:, 0:GW])
                    nc.sync.dma_start(tri_s[:], tri[:])
                    nc.sync.dma_start(qk[3][:, 0:GW], kt2[h][:, 0:GW])
                    nc.sync.dma_start(qk[2][:, 0:GW], qt2[h][:, 0:GW])
                    nc.gpsimd.dma_start(ones_mat[:], onesd[:])
                    nc.gpsimd.dma_start(neglam_s[:], neglam[:])
                    if USE_FP8:
                        nc.gpsimd.dma_start(ones8_s[:], ones8d[:])
                    nc.sync.dma_start(v_s[:, 0:TPG, :], vd[h][:, 0:TPG, :])
                    nc.gpsimd.dma_start(qk[1][:, GW:], kt1[h][:, GW:])
                    nc.gpsimd.dma_start(qk[3][:, GW:], kt2[h][:, GW:])
                    nc.gpsimd.dma_start(v_s[:, TPG:, :], vd[h][:, TPG:, :])
                    nc.gpsimd.dma_start(qk[0][:, GW:], qt1[h][:, GW:])
                    nc.gpsimd.dma_start(qk[2][:, GW:], qt2[h][:, GW:])
                    if USE_FP8:
                        nc.gpsimd.dma_start(v8_s[:], v8d[h])
                else:
                    # hoisted a full group (~14us) ahead of first use
                    nc.gpsimd.dma_start(qk[1][:], kt1[h][:])
                    nc.gpsimd.dma_start(qk[0][:], qt1[h][:])
                    nc.gpsimd.dma_start(qk[3][:], kt2[h][:])
                    nc.gpsimd.dma_start(qk[2][:], qt2[h][:])
                    nc.gpsimd.dma_start(v_s[:], vd[h])
                    if USE_FP8:
                        nc.gpsimd.dma_start(v8_s[:], v8d[h])
                head_tiles[h] = (qk, v_s, v8_s)

            # flat unit list across heads/groups; each unit = (score tile,
            # pass).  Group units: jfull//2 full pairs + 2 packed diag tiles.
            units = []
            for h in range(HEADS):
                for g in range(G):
                    gu = [("full", (2 * jp, 2 * jp + 1)) for jp in range(TPG * g // 2)]
                    gu += [("diag", (0, 1)), ("diag", (2, 3))]
                    for kind, js in gu:
                        for pi in range(2):
                            units.append((h, g, kind, js, pi))
            n_units = len(units)
            first_of_group = {}
            last_of_pass = {}
            for idx, (h, g, kind, js, pi) in enumerate(units):
                first_of_group.setdefault((h, g), idx)
                last_of_pass[(h, g, pi)] = idx

            ctxs = {}

            def stage1(idx):
                h, g, kind, js, pi = units[idx]
                if idx == first_of_group.get((h, 0), -1):
                    emit_loads(h)
                if h + 1 < HEADS and idx == first_of_group[(h, G - 1)]:
                    emit_loads(h + 1)  # prefetch next head early
                if (h, g) not in ctxs:
                    ctxs[(h, g)] = {
                        "outp": [
                            opool.tile([P, GW], fp32, tag=f"outp{p_}",
                                       name=f"outp{p_}_{h}_{g}")
                            for p_ in range(2)
                        ],
                        "sums": [
                            upool.tile([P, GW], fp32, tag=f"sums{p_}",
                                       name=f"sums{p_}_{h}_{g}")
                            for p_ in range(2)
                        ],
                        "t": [None, None],
                    }
                qk, v_s, v8_s = head_tiles[h]
                jfull = TPG * g
                st = spool.tile([P, 2 * GW], fp32, tag=f"st{pi}")
                if USE_FP8 and kind == "full":
                    et = epool.tile([P, 2 * GW], fp8, tag=f"et8{pi}")
                else:
                    et = epool.tile([P, 2 * GW], fp16, tag=f"et{pi}")
                ks = qk[2 * pi + 1]
                if kind == "full":
                    for jj, j in enumerate(js):
                        nc.tensor.matmul(
                            st[:, jj * GW : (jj + 1) * GW],
                            ks[:, j * P : (j + 1) * P],
                            qk[2 * pi][:, g * GW : (g + 1) * GW],
                            start=True,
                            stop=True,
                        )
                    regions = [(j, None, 0, GW, jj * GW) for jj, j in enumerate(js)]
                    width = 2 * GW
                else:
                    blk = GW - js[0] * P  # width of first region
                    regions = []
                    off = 0
                    for dr in js:
                        j = jfull + dr
                        col0 = dr * P      # q offset in group
                        n = GW - col0
                        regions.append((j, dr, col0, n, off))
                        nc.tensor.matmul(
                            st[:, off : off + n],
                            ks[:, j * P : (j + 1) * P],
                            qk[2 * pi][:, g * GW + col0 : (g + 1) * GW],
                            start=True,
                            stop=True,
                        )
                        off += blk
                    # causal band: first 128 cols of each region (regions
                    # start at 0 and blk -> uniform stride)
                    bands = st[:, 0 : 2 * blk].rearrange(
                        "p (b c) -> p b c", b=2, c=blk
                    )[:, :, 0:P]
                    nc.vector.copy_predicated(
                        bands,
                        tri_s[:].rearrange("p c -> p () c").broadcast_to([P, 2, P]),
                        negbig[:].rearrange("p c -> p () c").broadcast_to([P, 2, P]),
                    )
                    width = blk + (GW - js[1] * P)
                nc.scalar.activation(
                    et[:, :width], st[:, :width], Exp, scale=SCALE, bias=expb_s[:]
                )
                return (idx, et, regions)

            def stage2(idx, et, regions):
                h, g, kind, js, pi = units[idx]
                ctx = ctxs[(h, g)]
                jfull = TPG * g
                _, v_s, v8_s = head_tiles[h]
                if USE_FP8 and kind == "full":
                    # one DoubleRow matmul per pair: lhsT [128,2,128] fp8
                    # (two packed key-tiles), rhs [128,2,512] fp8 -> the PE
                    # streams both tiles' contraction in 512 columns.
                    jp = js[0] // 2
                    rhs3 = et.rearrange("p (b c) -> p b c", b=2, c=GW)
                    strt = js[0] == 0
                    nc.tensor.matmul(
                        ctx["sums"][pi][:], ones8_s[:], rhs3,
                        start=strt, stop=False, perf_mode=DR,
                    )
                    nc.tensor.matmul(
                        ctx["outp"][pi][:], v8_s[:, jp, :, :], rhs3,
                        start=strt, stop=False, perf_mode=DR,
                    )
                else:
                    for j, dr, col0, n, roff in regions:
                        ecols = et[:, roff : roff + n]
                        if kind == "full":
                            strt, stp = (j == 0), False
                        else:
                            strt = (dr == 0 and jfull == 0)
                            stp = (dr == TPG - 1)
                        nc.tensor.matmul(
                            ctx["sums"][pi][:, col0:], ones_mat[:], ecols,
                            start=strt, stop=stp,
                        )
                        nc.tensor.matmul(
                            ctx["outp"][pi][:, col0:], v_s[:, j, :], ecols,
                            start=strt, stop=stp,
                        )
                # per-pass epilogue half as soon as this pass's accumulation
                # is complete; combine + store once both halves are done
                if idx == last_of_pass[(h, g, pi)]:
                    rcp = fpool.tile([P, GW], fp32, tag=f"rcp{pi}")
                    scr = fpool.tile([P, GW], fp32, tag=f"scr{pi}")
                    nc.vector.reciprocal_approx_accurate(
                        rcp[:], ctx["sums"][pi][:], scr[:]
                    )
                    t_ = fpool.tile([P, GW], fp32, tag=f"t{pi}")
                    nc.vector.tensor_mul(t_[:], ctx["outp"][pi][:], rcp[:])
                    ctx["t"][pi] = t_
                    if all(x is not None for x in ctx["t"]):
                        fin = fpool.tile([P, GW], fp32, tag="fin")
                        # fin = t0 - lam*t1 (lam exact in fp32).  The very
                        # last group's combine+store is the serial tail of
                        # the kernel: split it in halves so the second stt
                        # overlaps the first store's DMA.
                        halves = (
                            (slice(0, GW // 2), slice(GW // 2, GW))
                            if (h, g) == (HEADS - 1, G - 1)
                            else (slice(0, GW),)
                        )
                        for sl_ in halves:
                            nc.vector.scalar_tensor_tensor(
                                fin[:, sl_], ctx["t"][1][:, sl_], neglam_s[:],
                                ctx["t"][0][:, sl_], op0=Mult, op1=Add,
                            )
                            gofs = g * GW
                            nc.sync.dma_start(
                                out[h][:, gofs + sl_.start : gofs + sl_.stop],
                                fin[:, sl_],
                            )
                        del ctxs[(h, g)]

            def want_pop(pend):
                # the first two stage2s of every group write PSUM banks the
                # previous group's epilogue is still reading; hold them 2
                # extra units so the reciprocal/multiply can release the
                # banks without stalling the PE
                if not pend:
                    return False
                idx0 = pend[0][0]
                h0, g0 = units[idx0][0], units[idx0][1]
                pos = idx0 - first_of_group[(h0, g0)]
                extra = 2 if (pos < 2 and (h0, g0) != (0, 0)) else 0
                return len(pend) > LAG + extra

            pend = []
            for idx in range(n_units):
                pend.append(stage1(idx))
                while want_pop(pend):
                    stage2(*pend.pop(0))
            for u in pend:
                stage2(*u)

    nc.compile()
    return nc


def _get_program():
    global _PROGRAM
    if _PROGRAM is None:
        _PROGRAM = _build_program()
    return _PROGRAM


def _make_in_maps(q1, k1, v, q2, k2, lambda_log):
    lam_val = float(np.exp(np.float64(lambda_log.reshape(-1)[0])))
    neglam_np = np.full((P, 1), -lam_val, dtype=np.float32)
    ones_np = np.ones((P, P), dtype=np.float16)
    # kill-mask for the diagonal band: 1 where k > q (strictly below diag)
    tri_np = (np.arange(P)[:, None] > np.arange(P)[None, :]).astype(np.uint8)

    def t(x, dt_):  # [BH, S, D] -> [BH, D, S] contiguous
        return np.ascontiguousarray(
            x.reshape(BH, S, D).transpose(0, 2, 1)
        ).astype(dt_)

    q1t = t(q1, np.float16)
    q2t = t(q2, np.float16)
    k1t = t(k1, np.float16)
    k2t = t(k2, np.float16)
    # pre-tile V to [BH, p, j, d] so the SBUF load is contiguous per
    # partition: v_s[p, j, d] = V[128 j + p, d]
    vr = v.reshape(BH, NT, P, D)
    vf = np.ascontiguousarray(vr.transpose(0, 2, 1, 3)).astype(np.float16)
    if USE_FP8:
        import ml_dtypes

        f8 = ml_dtypes.float8_e4m3
        # key-tile PAIRS for DoubleRow: v8[p, jp, ko, d] = V[128*(2jp+ko)+p, d]
        v8 = np.ascontiguousarray(
            vr[:, : 2 * V8_PAIRS].reshape(BH, V8_PAIRS, 2, P, D)
            .transpose(0, 3, 1, 2, 4)
        ).astype(f8)
        ones8_np = np.ones((P, 2, P), dtype=f8)

    in_maps = []
    for c in range(NCORES):
        sl = slice(c * HEADS, (c + 1) * HEADS)
        m = {
            "qt1": q1t[sl],
            "kt1": k1t[sl],
            "qt2": q2t[sl],
            "kt2": k2t[sl],
            "v": vf[sl],
            "neglam": neglam_np,
            "ones": ones_np,
            "tri": tri_np,
        }
        if USE_FP8:
            m["v8"] = v8[sl]
            m["ones8"] = ones8_np
        in_maps.append(m)
    return in_maps


def _run(q1, k1, v, q2, k2, lambda_log, trace=False):
    from concourse.bass_utils import run_bass_kernel_spmd

    nc = _get_program()
    in_maps = _make_in_maps(q1, k1, v, q2, k2, lambda_log)
    res = run_bass_kernel_spmd(
        nc, in_maps, core_ids=list(range(NCORES)), trace=trace
    )
    parts = [res.results[c]["out"].transpose(0, 2, 1) for c in range(NCORES)]
    full = np.concatenate(parts, axis=0).reshape(B, H, S, D)
    return np.ascontiguousarray(full, dtype=np.float32), res


def kernel(q1, k1, v, q2, k2, lambda_log):
    out, _ = _run(q1, k1, v, q2, k2, lambda_log, trace=False)
    return out
